# revision 4
# baseline (speedup 1.0000x reference)
"""Causal self-attention (B=2, T=2048, C=1024, H=16 heads, D=64) on 8 TRN2 NeuronCores.

Sharding: core c in 0..7 handles batch b = c//4 and heads [4*(c%4), 4*(c%4)+4).
Each core is fully independent (no collectives); host slices inputs / concatenates
outputs.

Per-core layout strategy:
  - hidden_states[b] is passed TRANSPOSED (C-major, bf16) so the contraction
    dim C of the QKV projections lands on SBUF partitions with no on-device
    transposes.
  - Q and K are produced directly in transposed form [d, t] (d on partitions,
    two heads stacked -> 128 partitions), which is exactly the layout the
    score matmul S^T = K_chunk^T-free ... needs:
        S^T[j, i] = sum_d KT[d, j] * QT[d, i]   (lhsT = KT chunk, rhs = QT chunk)
    The two heads of a pair sit at base partitions 0 and 64, so consecutive
    K=64 matmuls occupy disjoint PE row groups and overlap in the array.
  - softmax: no max-subtraction needed (scores are O(1) by construction:
    exp overflow impossible), so E = exp(S/8 + attention_mask[j]) via one
    ScalarE activation (scale/bias folded in). Causality: matmuls only cover
    the lower triangle (narrowed on diagonal tiles) plus one 128x128
    triangular corner mask multiply per diagonal tile.
  - V is stored [t, 64+1] with a ones-column, so O^T = V^T @ E accumulation in
    PSUM also accumulates the softmax denominator in output row 64.
  - Epilogue per 128-column chunk: PE-transpose [65, 128] -> [128, 65]
    (queries back on partitions), reciprocal of the sums column, per-partition
    scalar multiply -> normalized output chunk, staged and DMA'd out.
"""

import os
import sys

sys.path.insert(0, "/opt/trn_rl_repo")

import numpy as np
import ml_dtypes

import concourse.bass as bass
import concourse.tile as tile
from concourse import bacc, mybir
from concourse.bass_utils import run_bass_kernel_spmd

B, T, C, H, D = 2, 2048, 1024, 16, 64
P = 128
KO = C // P           # 8 k-subtiles for projections
NCORES = 8
HPC = 4               # heads per core
CPC = HPC * D         # output channels per core = 256
NPAIR = HPC // 2      # head pairs per core
NTB = T // P          # 16 t-blocks / j-tiles
NW = 2                # i-windows per row
WW = T // NW          # window width = 1024

f32 = mybir.dt.float32
bf16 = mybir.dt.bfloat16
AF = mybir.ActivationFunctionType
ALU = mybir.AluOpType

_COMPILED = None


def _build_kernel():
    nc = bacc.Bacc("TRN2", target_bir_lowering=False, debug=False)

    xt_d = nc.dram_tensor("xt", [C, T], bf16, kind="ExternalInput").ap()
    wq_d = nc.dram_tensor("wq", [C, CPC], bf16, kind="ExternalInput").ap()
    wk_d = nc.dram_tensor("wk", [C, CPC], bf16, kind="ExternalInput").ap()
    wv_d = nc.dram_tensor("wv", [C, CPC], bf16, kind="ExternalInput").ap()
    bq_d = nc.dram_tensor("bq", [CPC], f32, kind="ExternalInput").ap()
    bk_d = nc.dram_tensor("bk", [CPC], f32, kind="ExternalInput").ap()
    bv_d = nc.dram_tensor("bv", [CPC], f32, kind="ExternalInput").ap()
    am_d = nc.dram_tensor("am", [T], f32, kind="ExternalInput").ap()
    out_d = nc.dram_tensor("out", [T, CPC], f32, kind="ExternalOutput").ap()

    # constants baked into the NEFF
    tri_np = np.triu(np.ones((P, P), np.float32)).astype(ml_dtypes.bfloat16)
    tri_d = nc.inline_tensor(tri_np, "tri").ap()
    id_np = np.eye(P, dtype=np.float32)
    id_d = nc.inline_tensor(id_np, "ident").ap()

    with tile.TileContext(nc) as tc:
        _kernel_body(tc, xt_d, wq_d, wk_d, wv_d, bq_d, bk_d, bv_d, am_d,
                     tri_d, id_d, out_d)

    nc.compile()
    return nc


def _kernel_body(tc, xt_d, wq_d, wk_d, wv_d, bq_d, bk_d, bv_d, am_d,
                 tri_d, id_d, out_d):
    nc = tc.nc

    with (
        tc.tile_pool(name="const", bufs=1) as const_pool,
        tc.tile_pool(name="qk", bufs=1) as qk_pool,
        tc.tile_pool(name="v", bufs=1) as v_pool,
        tc.tile_pool(name="e", bufs=3) as e_pool,
        tc.tile_pool(name="ot", bufs=2) as ot_pool,
        tc.tile_pool(name="stage", bufs=2) as stage_pool,
        tc.tile_pool(name="rcp", bufs=8) as rcp_pool,
        tc.tile_pool(name="ps", bufs=4, space="PSUM") as ps_pool,
    ):
        # ---- constant / input loads -------------------------------------
        xt_sb = const_pool.tile([P, KO, T], bf16)
        xt_r = xt_d.rearrange("(o p) t -> p o t", p=P)
        for o in range(KO):
            nc.sync.dma_start(xt_sb[:, o, :], xt_r[:, o, :])

        wq_sb = const_pool.tile([P, KO, CPC], bf16)
        nc.sync.dma_start(wq_sb[:], wq_d.rearrange("(o p) d -> p o d", p=P))
        wk_sb = const_pool.tile([P, KO, CPC], bf16)
        nc.sync.dma_start(wk_sb[:], wk_d.rearrange("(o p) d -> p o d", p=P))
        wv_sb = const_pool.tile([P, KO, CPC], bf16)
        nc.sync.dma_start(wv_sb[:], wv_d.rearrange("(o p) d -> p o d", p=P))

        bq_sb = const_pool.tile([P, NPAIR], f32)
        nc.sync.dma_start(bq_sb[:], bq_d.rearrange("(a p) -> p a", p=P))
        bk_sb = const_pool.tile([P, NPAIR], f32)
        nc.sync.dma_start(bk_sb[:], bk_d.rearrange("(a p) -> p a", p=P))
        # bv is added inside the V-projection PSUM accumulation via a K=1
        # matmul: psv += ones[1,128].T @ bv[1,256]  (broadcast add over t)
        bv_sb = const_pool.tile([1, CPC], bf16)
        nc.gpsimd.dma_start(bv_sb[:], bv_d[None, :])
        ones_sb = const_pool.tile([1, P], bf16)
        nc.vector.memset(ones_sb[:], 1.0)

        am_sb = const_pool.tile([P, NTB], f32)
        nc.sync.dma_start(am_sb[:], am_d.rearrange("(a p) -> p a", p=P))

        tri_sb = const_pool.tile([P, P], bf16)
        nc.sync.dma_start(tri_sb[:], tri_d)
        id_sb = const_pool.tile([P, P], f32)
        nc.sync.dma_start(id_sb[:], id_d)

        # ---- phase 1: projections --------------------------------------
        # Q^T and K^T per head pair: [128 (= 2 heads x 64 d), T] bf16
        qt_sb = qk_pool.tile([P, NPAIR, T], bf16)
        kt_sb = qk_pool.tile([P, NPAIR, T], bf16)
        # V per head: [128 j, NTB, 65] bf16 with ones column at 64
        v_sb = v_pool.tile([P, HPC, NTB, D + 1], bf16)
        nc.vector.memset(v_sb[:, :, :, D:D + 1], 1.0)

        NQ = T // 512  # 4 n-chunks of 512
        for pair in range(NPAIR):
            for n in range(NQ):
                ps = ps_pool.tile([P, WW], f32, tag="ps")
                psq = ps[:, 0:512]
                for ko in range(KO):
                    nc.tensor.matmul(
                        psq,
                        lhsT=wq_sb[:, ko, pair * P:(pair + 1) * P],
                        rhs=xt_sb[:, ko, n * 512:(n + 1) * 512],
                        start=(ko == 0), stop=(ko == KO - 1),
                    )
                nc.scalar.activation(
                    qt_sb[:, pair, n * 512:(n + 1) * 512], psq,
                    AF.Identity, bias=bq_sb[:, pair:pair + 1], scale=1.0,
                )
            for n in range(NQ):
                ps = ps_pool.tile([P, WW], f32, tag="ps")
                psk = ps[:, 0:512]
                for ko in range(KO):
                    nc.tensor.matmul(
                        psk,
                        lhsT=wk_sb[:, ko, pair * P:(pair + 1) * P],
                        rhs=xt_sb[:, ko, n * 512:(n + 1) * 512],
                        start=(ko == 0), stop=(ko == KO - 1),
                    )
                nc.scalar.activation(
                    kt_sb[:, pair, n * 512:(n + 1) * 512], psk,
                    AF.Identity, bias=bk_sb[:, pair:pair + 1], scale=1.0,
                )

        for tt in range(NTB):
            ps = ps_pool.tile([P, WW], f32, tag="ps")
            psv = ps[:, 0:CPC]
            for ko in range(KO):
                nc.tensor.matmul(
                    psv,
                    lhsT=xt_sb[:, ko, tt * P:(tt + 1) * P],
                    rhs=wv_sb[:, ko, :],
                    start=(ko == 0), stop=False,
                )
            nc.tensor.matmul(
                psv, lhsT=ones_sb[:], rhs=bv_sb[:], start=False, stop=True,
            )
            nc.vector.tensor_copy(
                v_sb[:, :, tt, 0:D],
                psv.rearrange("p (h d) -> p h d", h=HPC),
            )

        # ---- phase 2: attention ----------------------------------------
        for pair in range(NPAIR):
            stage = stage_pool.tile([P, NTB, P], f32)
            for hh in range(2):
                h = pair * 2 + hh
                dlo, dhi = hh * D, (hh + 1) * D
                for it2 in range(NW):
                    w0 = WW * it2
                    jt_max = (w0 + WW) // P  # j-tiles 0 .. jt_max-1
                    pv_full = ps_pool.tile([P, WW], f32, tag="ps", name="pv")
                    pv = pv_full[0:D + 1, :]
                    for jt in range(jt_max):
                        s = max(0, P * jt - w0)  # window-local start col
                        st = ps_pool.tile([P, WW], f32, tag="ps")
                        ranges = []
                        if s < 512:
                            ranges.append((s, 512))
                        ranges.append((max(s, 512), WW))
                        for (a, b) in ranges:
                            nc.tensor.matmul(
                                st[:, a:b],
                                lhsT=kt_sb[dlo:dhi, pair, jt * P:(jt + 1) * P],
                                rhs=qt_sb[dlo:dhi, pair, w0 + a:w0 + b],
                                start=True, stop=True,
                            )
                        e = e_pool.tile([P, WW], bf16)
                        nc.scalar.activation(
                            e[:, s:WW], st[:, s:WW], AF.Exp,
                            bias=am_sb[:, jt:jt + 1], scale=0.125,
                        )
                        if P * jt >= w0:  # diagonal tile: triangular corner
                            nc.vector.tensor_tensor(
                                e[:, s:s + P], e[:, s:s + P], tri_sb, ALU.mult,
                            )
                        for (a, b) in ranges:
                            last = (jt == jt_max - 1) if b == WW else \
                                (jt == (w0 + 512) // P - 1)
                            nc.tensor.matmul(
                                pv[:, a:b],
                                lhsT=v_sb[:, h, jt, :],
                                rhs=e[:, a:b],
                                start=(jt == 0), stop=last,
                            )
                    # epilogue for (h, it2)
                    ot = ot_pool.tile([D + 1, WW], f32)
                    nc.vector.tensor_copy(ot[:], pv)
                    for ci in range(WW // P):
                        tp = ps_pool.tile([P, WW], f32, tag="ps")
                        nc.tensor.transpose(
                            tp[:, 0:D + 1],
                            ot[:, ci * P:(ci + 1) * P],
                            id_sb[0:D + 1, 0:D + 1],
                        )
                        rc = rcp_pool.tile([P, 1], f32)
                        nc.vector.reciprocal(rc, tp[:, D:D + 1])
                        nc.vector.tensor_scalar_mul(
                            stage[:, it2 * (WW // P) + ci, dlo:dhi],
                            tp[:, 0:D], rc,
                        )
            nc.sync.dma_start(
                out_d.rearrange("(tb p) c -> p tb c", p=P)[:, :, pair * P:(pair + 1) * P],
                stage[:],
            )


def _get_compiled():
    global _COMPILED
    if _COMPILED is None:
        _COMPILED = _build_kernel()
    return _COMPILED


def _make_in_maps(hidden_states, attention_mask, Wq, bq, Wk, bk, Wv, bv):
    X = np.asarray(hidden_states, dtype=np.float32)
    AM = np.asarray(attention_mask, dtype=np.float32)
    in_maps = []
    for core in range(NCORES):
        b = core // 4
        hp = core % 4
        rows = slice(hp * CPC, (hp + 1) * CPC)
        in_maps.append({
            "xt": np.ascontiguousarray(X[b].T).astype(ml_dtypes.bfloat16),
            "wq": np.ascontiguousarray(np.asarray(Wq)[rows].T).astype(ml_dtypes.bfloat16),
            "wk": np.ascontiguousarray(np.asarray(Wk)[rows].T).astype(ml_dtypes.bfloat16),
            "wv": np.ascontiguousarray(np.asarray(Wv)[rows].T).astype(ml_dtypes.bfloat16),
            "bq": np.ascontiguousarray(np.asarray(bq, dtype=np.float32)[rows]),
            "bk": np.ascontiguousarray(np.asarray(bk, dtype=np.float32)[rows]),
            "bv": np.ascontiguousarray(np.asarray(bv, dtype=np.float32)[rows]),
            "am": np.ascontiguousarray(AM[b, 0, 0, :]),
        })
    return in_maps


def _gather(results):
    out = np.empty((B, T, C), dtype=np.float32)
    for core in range(NCORES):
        b = core // 4
        hp = core % 4
        out[b, :, hp * CPC:(hp + 1) * CPC] = results[core]["out"]
    return out


def run(trace=False, **inputs):
    nc = _get_compiled()
    in_maps = _make_in_maps(**inputs)
    res = run_bass_kernel_spmd(nc, in_maps, list(range(NCORES)), trace=trace)
    return _gather(res.results), res


def kernel(**inputs):
    out, _ = run(trace=False, **inputs)
    return out
